# revision 42
# baseline (speedup 1.0000x reference)
"""Distributed Trainium2 kernel for GNN message passing (COO SpMM + dense head).

out = relu((A @ x) @ W[:128] + x @ W[128:])   with A given as COO (rows, cols, vals)

Strategy (8 NeuronCores, SPMD single graph):
  - Rows (destinations) sharded across cores: core c owns rows [c*12500, (c+1)*12500).
  - x is replicated to every core's DRAM via its input map (no collectives needed).
  - Host-side index preprocessing only (sorting / padding / layout): per core the
    edge list is sorted by col-chunk (4 chunks of 25000 so gather indices fit in
    int16), padded to shared per-chunk capacities so all 8 cores run the same graph.
  - On device per tile of edges: hardware gather x[col] (SWDGE dma_gather),
    scale by vals (VectorE broadcast multiply), hardware scatter-add into a DRAM
    h accumulator (SWDGE dma_scatter_add).  Paired edge occurrences are laid out
    so one 1KB scatter descriptor writes both parity stripes of h[row] at once,
    cutting Q7 descriptor-generation (the measured bottleneck, ~7ns/descriptor)
    from 153K to ~127K descriptors per core.
  - Dense head (batched 4 groups per DMA): combine h stripes, h.T via PE
    transpose, out = relu(hT.T@W1 + xT.T@W2) accumulated in PSUM, relu on
    ScalarE; the xlocT arena is preloaded to SBUF during the SpMM phase.
"""

import sys

if "/opt/trn_rl_repo" not in sys.path:
    sys.path.insert(0, "/opt/trn_rl_repo")

import numpy as np

N_NODES = 100000
N_EDGES = 600000
D = 128
OUT = 128
P = 128
NCORES = 8
RPC = N_NODES // NCORES          # 12500 rows per core
NCHUNK = 4
CHUNK = N_NODES // NCHUNK        # 25000 (< 32768 so int16 gather idx works)
TILE_E = 1024                    # max edges per SWDGE call (1024-descriptor ring limit)
H_PAD = 12800                    # padded row count per parity stripe
MAXROUNDS = 16                   # upper bound on per-(chunk,row) multiplicity / 2

_compiled = {}


def _prep(adj_rows, adj_cols, adj_vals):
    """Per-core uniform-shape gather/scatter metadata (int/layout work only).

    Round j of a (chunk,row) group holds edge occurrences {2j, 2j+1}.  Rows
    with both occurrences present ("paired") are laid out so occurrence 2j
    sits at position b*256+m and 2j+1 at b*256+128+m (same SBUF partition,
    adjacent 128-slots) - one 1KB scatter descriptor then writes both parity
    stripes of h2[row] at once (elem_size=256).  Rows with only occurrence 2j
    ("unpaired") go in separate 512B-token calls into the [25600,128] view.
    Within any call destinations are unique, calls are WAW-serialized, so the
    non-atomic HW scatter-add never races.
    """
    rows = np.asarray(adj_rows).astype(np.int64)
    cols = np.asarray(adj_cols).astype(np.int64)
    vals = np.asarray(adj_vals).astype(np.float32)

    per_core = []
    # sizes[c, chunk, round, type]: type 0 = paired (edge count), 1 = unpaired
    sizes_all = np.zeros((NCORES, NCHUNK, MAXROUNDS, 2), np.int64)
    for c in range(NCORES):
        m = (rows >= c * RPC) & (rows < (c + 1) * RPC)
        r = rows[m] - c * RPC
        co = cols[m]
        v = vals[m]
        ch = co // CHUNK
        o = np.lexsort((r, ch))
        r, co, v, ch = r[o], co[o], v[o], ch[o]
        key = ch * RPC + r
        n = len(key)
        change = np.empty(n, bool)
        if n:
            change[0] = True
            change[1:] = key[1:] != key[:-1]
        starts = np.flatnonzero(change)
        lens = np.diff(np.append(starts, n))
        occ = np.arange(n) - np.repeat(starts, lens)
        kcnt = np.repeat(lens, lens)          # per-edge group size
        rnd = occ // 2
        assert rnd.max(initial=0) < MAXROUNDS
        parity = occ % 2
        paired = (2 * rnd + 1) < kcnt         # partner exists
        typ = (~paired).astype(np.int64)      # 0 = paired, 1 = unpaired
        # order: (chunk, round, type, parity, row) -> within a paired group the
        # parity-0 run and parity-1 run list the same rows in the same order
        o2 = np.lexsort((r, parity, typ, rnd, ch))
        r, co, v, ch, rnd, parity, typ = (
            a[o2] for a in (r, co, v, ch, rnd, parity, typ))
        for k in range(NCHUNK):
            for t in range(2):
                mk = (ch == k) & (typ == t)
                sizes_all[c, k, :, t] += np.bincount(
                    rnd[mk], minlength=MAXROUNDS)
        per_core.append((r, co % CHUNK, v, ch, rnd, parity, typ))

    caps = np.zeros((NCHUNK, MAXROUNDS, 2), np.int64)
    caps[:, :, 0] = ((sizes_all[:, :, :, 0].max(axis=0) + 255) // 256) * 256
    caps[:, :, 1] = ((sizes_all[:, :, :, 1].max(axis=0) + 127) // 128) * 128
    T = int(caps.sum())

    # static call list: (chunk, dram_offset, n_edges, type)
    calls = []
    off = 0
    for k in range(NCHUNK):
        for j in range(MAXROUNDS):
            for t in range(2):
                cap = int(caps[k, j, t])
                tile_sz = 2048 if t == 0 else TILE_E
                for t0 in range(0, cap, tile_sz):
                    calls.append((k, off + t0, min(tile_sz, cap - t0), t))
                off += cap

    gidx_w = np.zeros((NCORES, P, T // 16), np.int16)
    sp_w = np.zeros((NCORES, P, T // 2 // 16), np.int16)   # paired: idx per pair
    su_w = np.zeros((NCORES, P, T // 16), np.int16)        # unpaired idx
    sval_w = np.zeros((NCORES, P, T // 128), np.float32)
    offs = np.concatenate([[0], np.cumsum(caps.reshape(-1))]).astype(np.int64)

    for c in range(NCORES):
        r, coi, v, ch, rnd, parity, typ = per_core[c]
        gi = np.zeros(T, np.int16)
        sp = np.full(T // 2, 12600, np.int16)   # paired dump row (1KB units)
        su = np.full(T, 25200, np.int16)        # unpaired dump (512B units)
        sv = np.zeros(T, np.float32)
        s = 0
        gidx = 0
        for k in range(NCHUNK):
            for j in range(MAXROUNDS):
                for t in range(2):
                    d0 = int(offs[gidx]); gidx += 1
                    nn = int(sizes_all[c, k, j, t])
                    if t == 1:
                        gi[d0:d0 + nn] = coi[s:s + nn]
                        su[d0:d0 + nn] = r[s:s + nn] * 2   # parity always 0
                        sv[d0:d0 + nn] = v[s:s + nn]
                        s += nn
                        continue
                    # paired: nn edges = 2*npair; first npair are parity 0
                    npair = nn // 2
                    mseq = np.arange(npair)
                    pos0 = d0 + (mseq // 128) * 256 + (mseq % 128)
                    pos1 = pos0 + 128
                    gi[pos0] = coi[s:s + npair]
                    sv[pos0] = v[s:s + npair]
                    gi[pos1] = coi[s + npair:s + nn]
                    sv[pos1] = v[s + npair:s + nn]
                    sp[(d0 // 2) + mseq] = r[s:s + npair]
                    s += nn
        gidx_w[c] = np.tile(gi.reshape(-1, 16).T, (8, 1))
        sp_w[c] = np.tile(sp.reshape(-1, 16).T, (8, 1))
        su_w[c] = np.tile(su.reshape(-1, 16).T, (8, 1))
        sval_w[c] = sv.reshape(-1, 128).T

    return tuple(calls), gidx_w, sp_w, su_w, sval_w


def _build(calls):
    from concourse import bass, mybir, tile, bacc
    from concourse.masks import make_identity

    f32 = mybir.dt.float32
    i16 = mybir.dt.int16
    T = max(e0 + n for _, e0, n, _t in calls)

    nc = bacc.Bacc("TRN2", target_bir_lowering=False, debug=False,
                   num_swdge_queues=4)

    x_d = nc.dram_tensor("x", [N_NODES, D], f32, kind="ExternalInput")
    xT_d = nc.dram_tensor("xlocT", [D, RPC], f32, kind="ExternalInput")
    w_d = nc.dram_tensor("W", [2 * D, OUT], f32, kind="ExternalInput")
    gidx_d = nc.dram_tensor("gidx", [P, T // 16], i16, kind="ExternalInput")
    sp_d = nc.dram_tensor("spair", [P, T // 2 // 16], i16, kind="ExternalInput")
    su_d = nc.dram_tensor("sunp", [P, T // 16], i16, kind="ExternalInput")
    sval_d = nc.dram_tensor("svals", [P, T // 128], f32, kind="ExternalInput")
    out_d = nc.dram_tensor("out", [RPC, OUT], f32, kind="ExternalOutput")
    h_d = nc.dram_tensor("h_acc", [H_PAD, 2 * D], f32)   # 1KB rows, 2 stripes

    relu = mybir.ActivationFunctionType.Relu

    with tile.TileContext(nc) as tc:
        with tc.tile_pool(name="const", bufs=1) as constp, \
             tc.tile_pool(name="mess", bufs=6) as messp, \
             tc.tile_pool(name="meta", bufs=6) as metap, \
             tc.tile_pool(name="dense", bufs=4) as densep, \
             tc.tile_pool(name="psum", bufs=2, space="PSUM") as psump:

            ident = constp.tile([P, P], f32)
            make_identity(nc, ident[:])
            w1 = constp.tile([D, OUT], f32)
            nc.sync.dma_start(out=w1[:], in_=w_d[:D, :])
            w2 = constp.tile([D, OUT], f32)
            nc.sync.dma_start(out=w2[:], in_=w_d[D:, :])
            # preload the whole xlocT arena (tail's x-side inputs, h-independent)
            xta = constp.tile([P, RPC], f32)
            for c0 in range(0, RPC, 2500):
                nc.scalar.dma_start(out=xta[:, c0:c0 + 2500],
                                    in_=xT_d[:, c0:c0 + 2500])

            # zero the striped h accumulator (12800 rows x 1KB)
            zblk = constp.tile([P, 4, 2 * D], f32)
            nc.vector.memset(zblk[:], 0.0)
            for b in range(H_PAD // 512):
                dst = h_d[b * 512:(b + 1) * 512, :].rearrange(
                    "(a p) d -> p a d", p=P)
                nc.scalar.dma_start(out=dst, in_=zblk[:])

            # ---- SpMM phase: gather -> scale -> scatter-add ----
            h_unp = h_d[:].rearrange("r (s d) -> (r s) d", s=2)
            qrr = 0
            for k, e0, n, typ in calls:
                ns = n // 128
                x_chunk = x_d[k * CHUNK:(k + 1) * CHUNK, :]
                gi = metap.tile([P, 2048 // 16], i16, tag="gi")
                nc.sync.dma_start(
                    out=gi[:, :n // 16],
                    in_=gidx_d[:, e0 // 16:(e0 + n) // 16])
                mv = messp.tile([P, 2048 // 128, D], f32, tag="mess")
                for sub in range(0, n, 1024):
                    gn = min(1024, n - sub)
                    nc.gpsimd.dma_gather(
                        mv[:, sub // 128:(sub + gn) // 128, :], x_chunk,
                        gi[:, sub // 16:(sub + gn) // 16], gn, gn, D,
                        queue_num=1 + (qrr % 3))
                    qrr += 1
                sv = metap.tile([P, 2048 // 128], f32, tag="sv")
                nc.sync.dma_start(
                    out=sv[:, :ns],
                    in_=sval_d[:, e0 // 128:(e0 + n) // 128])
                nc.vector.tensor_tensor(
                    out=mv[:, :ns, :], in0=mv[:, :ns, :],
                    in1=sv[:, :ns, None].to_broadcast([P, ns, D]),
                    op=mybir.AluOpType.mult)
                if typ == 0:
                    npr = n // 2
                    si = metap.tile([P, 2048 // 2 // 16], i16, tag="sip")
                    nc.scalar.dma_start(
                        out=si[:, :npr // 16],
                        in_=sp_d[:, e0 // 2 // 16:(e0 // 2 + npr) // 16])
                    nc.gpsimd.dma_scatter_add(
                        h_d[:],
                        mv[:, :ns, :].rearrange("p a d -> p (a d)").rearrange(
                            "p (a d) -> p a d", d=2 * D),
                        si[:, :npr // 16], npr, npr, 2 * D)
                else:
                    si = metap.tile([P, TILE_E // 16], i16, tag="si")
                    nc.scalar.dma_start(
                        out=si[:, :n // 16],
                        in_=su_d[:, e0 // 16:(e0 + n) // 16])
                    nc.gpsimd.dma_scatter_add(
                        h_unp, mv[:, :ns, :], si[:, :n // 16], n, n, D)

            # ---- dense head: out = relu(h @ W1 + x @ W2) ----
            # batched: 4 row-groups (512 rows) per h-load / out-store DMA
            for b in range((RPC + 511) // 512):
                r0 = b * 512
                rb = min(512, RPC - r0)
                nsub = (rb + P - 1) // P
                hl4 = densep.tile([P, 4, 2 * D], f32, tag="hl")
                nc.sync.dma_start(
                    out=hl4[:, :nsub, :],
                    in_=h_d[r0:r0 + nsub * P, :].rearrange(
                        "(a p) d -> p a d", p=P))
                ob4 = densep.tile([P, 4, OUT], f32, tag="ob")
                for a in range(nsub):
                    g0 = r0 + a * P
                    rsz = min(P, RPC - g0)
                    hb = densep.tile([P, D], f32, tag="hb")
                    nc.vector.tensor_add(out=hb[:rsz, :],
                                         in0=hl4[:rsz, a, :D],
                                         in1=hl4[:rsz, a, D:])
                    pt = psump.tile([P, P], f32, tag="pt")
                    nc.tensor.transpose(pt[:, :rsz], hb[:rsz, :],
                                        ident[:rsz, :rsz])
                    hT = densep.tile([P, P], f32, tag="hT")
                    nc.vector.tensor_copy(hT[:, :rsz], pt[:, :rsz])
                    po = psump.tile([P, OUT], f32, tag="po")
                    nc.tensor.matmul(po[:rsz, :], hT[:, :rsz], w1[:],
                                     start=True, stop=False)
                    nc.tensor.matmul(po[:rsz, :], xta[:, g0:g0 + rsz], w2[:],
                                     start=False, stop=True)
                    nc.scalar.activation(ob4[:rsz, a, :], po[:rsz, :], relu)
                if rb == 512:
                    nc.scalar.dma_start(
                        out=out_d[r0:r0 + 512, :].rearrange(
                            "(a p) d -> p a d", p=P),
                        in_=ob4[:])
                else:
                    for a in range(nsub):
                        g0 = r0 + a * P
                        rsz = min(P, RPC - g0)
                        nc.scalar.dma_start(out=out_d[g0:g0 + rsz, :],
                                            in_=ob4[:rsz, a, :])

    nc.compile()
    return nc


def _get_nc(calls):
    nc = _compiled.get(calls)
    if nc is None:
        nc = _build(calls)
        _compiled[calls] = nc
    return nc


def _make_in_maps(x, W, calls, gidx_w, sp_w, su_w, sval_w):
    x = np.ascontiguousarray(np.asarray(x, np.float32))
    W = np.ascontiguousarray(np.asarray(W, np.float32))
    in_maps = []
    for c in range(NCORES):
        xloc = x[c * RPC:(c + 1) * RPC]
        in_maps.append({
            "x": x,
            "xlocT": np.ascontiguousarray(xloc.T),
            "W": W,
            "gidx": gidx_w[c],
            "spair": sp_w[c],
            "sunp": su_w[c],
            "svals": sval_w[c],
        })
    return in_maps


def _install_trace_shims():
    """Make trace=True work in this container: provide antenv.axon_hooks
    (ctypes NTFF profiling via the axon PJRT .so) and stub the artifact
    upload (no bucket access here)."""
    import contextlib
    import ctypes
    import types

    try:
        import antenv.axon_hooks  # noqa: F401
        has_hooks = True
    except ImportError:
        has_hooks = False
    if not has_hooks:
        so_path = "/opt/axon/libaxon_pjrt.so"
        lib = ctypes.CDLL(so_path)
        if hasattr(lib, "axon_start_nrt_profile"):
            lib.axon_start_nrt_profile.argtypes = [
                ctypes.POINTER(ctypes.c_int64), ctypes.c_size_t]
            lib.axon_start_nrt_profile.restype = ctypes.c_int64
            lib.axon_stop_nrt_profile.argtypes = [ctypes.c_char_p]
            lib.axon_stop_nrt_profile.restype = ctypes.c_int64

            @contextlib.contextmanager
            def _hook(output_dir, device_ids):
                import jax
                jax.devices()
                if device_ids:
                    ids = (ctypes.c_int64 * len(device_ids))(*device_ids)
                    rc = lib.axon_start_nrt_profile(ids, len(device_ids))
                else:
                    rc = lib.axon_start_nrt_profile(None, 0)
                if rc != 0:
                    raise RuntimeError(f"axon_start_nrt_profile rc={rc}")
                try:
                    yield
                finally:
                    n = lib.axon_stop_nrt_profile(str(output_dir).encode())
                    if n <= 0:
                        print(f"ntff profile: rc={n} (no files?) at {output_dir}")

            mod = types.ModuleType("antenv.axon_hooks")
            mod.get_axon_ntff_profile_hook = lambda: _hook
            mod.set_axon_ntff_profile_hook = lambda h: None
            sys.modules["antenv.axon_hooks"] = mod

    import concourse.bass_utils as bu
    bu.upload_artifacts = lambda tmpdir: f"local:{tmpdir}"


def _run(x, adj_rows, adj_cols, adj_vals, W, trace=False):
    from concourse.bass_utils import run_bass_kernel_spmd
    if trace:
        try:
            _install_trace_shims()
        except Exception as e:  # tracing is best-effort
            print("trace shim install failed:", e)
    calls, gidx_w, sp_w, su_w, sval_w = _prep(adj_rows, adj_cols, adj_vals)
    nc = _get_nc(calls)
    in_maps = _make_in_maps(x, W, calls, gidx_w, sp_w, su_w, sval_w)
    res = run_bass_kernel_spmd(nc, in_maps, list(range(NCORES)), trace=trace)
    out = np.concatenate([res.results[c]["out"] for c in range(NCORES)], axis=0)
    return out, res


def kernel(x, adj_rows, adj_cols, adj_vals, W):
    out, _ = _run(x, adj_rows, adj_cols, adj_vals, W, trace=False)
    return out


# revision 43
# speedup vs baseline: 1.1259x; 1.1259x over previous
"""Distributed Trainium2 kernel for GNN message passing (COO SpMM + dense head).

out = relu((A @ x) @ W[:128] + x @ W[128:])   with A given as COO (rows, cols, vals)

Strategy (8 NeuronCores, SPMD single graph):
  - Rows (destinations) sharded across cores: core c owns rows [c*12500, (c+1)*12500).
  - x is replicated to every core's DRAM via its input map (no collectives needed).
  - Host-side index preprocessing only (sorting / padding / layout): per core the
    edge list is sorted by col-chunk (4 chunks of 25000 so gather indices fit in
    int16), padded to shared per-chunk capacities so all 8 cores run the same graph.
  - On device per tile of edges: hardware gather x[col] (SWDGE dma_gather),
    scale by vals (VectorE broadcast multiply), hardware scatter-add into a DRAM
    h accumulator (SWDGE dma_scatter_add).  Paired edge occurrences are laid out
    so one 1KB scatter descriptor writes both parity stripes of h[row] at once,
    cutting Q7 descriptor-generation (the measured bottleneck, ~7ns/descriptor)
    from 153K to ~127K descriptors per core.
  - Dense head (batched 4 groups per DMA): combine h stripes, h.T via PE
    transpose, out = relu(hT.T@W1 + xT.T@W2) accumulated in PSUM, relu on
    ScalarE; the xlocT arena is preloaded to SBUF during the SpMM phase.
"""

import sys

if "/opt/trn_rl_repo" not in sys.path:
    sys.path.insert(0, "/opt/trn_rl_repo")

import numpy as np

N_NODES = 100000
N_EDGES = 600000
D = 128
OUT = 128
P = 128
NCORES = 8
RPC = N_NODES // NCORES          # 12500 rows per core
NCHUNK = 4
CHUNK = N_NODES // NCHUNK        # 25000 (< 32768 so int16 gather idx works)
TILE_E = 1024                    # max edges per SWDGE call (1024-descriptor ring limit)
H_PAD = 12800                    # padded row count per parity stripe
MAXROUNDS = 16                   # upper bound on per-(chunk,row) multiplicity / 2

_compiled = {}


def _prep(adj_rows, adj_cols, adj_vals):
    """Per-core uniform-shape gather/scatter metadata (int/layout work only).

    Round j of a (chunk,row) group holds edge occurrences {2j, 2j+1}.  Rows
    with both occurrences present ("paired") are laid out so occurrence 2j
    sits at position b*256+m and 2j+1 at b*256+128+m (same SBUF partition,
    adjacent 128-slots) - one 1KB scatter descriptor then writes both parity
    stripes of h2[row] at once (elem_size=256).  Rows with only occurrence 2j
    ("unpaired") go in separate 512B-token calls into the [25600,128] view.
    Within any call destinations are unique, calls are WAW-serialized, so the
    non-atomic HW scatter-add never races.
    """
    rows = np.asarray(adj_rows).astype(np.int64)
    cols = np.asarray(adj_cols).astype(np.int64)
    vals = np.asarray(adj_vals).astype(np.float32)

    per_core = []
    # sizes[c, chunk, round, type]: type 0 = paired (edge count), 1 = unpaired
    sizes_all = np.zeros((NCORES, NCHUNK, MAXROUNDS, 2), np.int64)
    for c in range(NCORES):
        m = (rows >= c * RPC) & (rows < (c + 1) * RPC)
        r = rows[m] - c * RPC
        co = cols[m]
        v = vals[m]
        ch = co // CHUNK
        o = np.lexsort((r, ch))
        r, co, v, ch = r[o], co[o], v[o], ch[o]
        key = ch * RPC + r
        n = len(key)
        change = np.empty(n, bool)
        if n:
            change[0] = True
            change[1:] = key[1:] != key[:-1]
        starts = np.flatnonzero(change)
        lens = np.diff(np.append(starts, n))
        occ = np.arange(n) - np.repeat(starts, lens)
        kcnt = np.repeat(lens, lens)          # per-edge group size
        rnd = occ // 2
        assert rnd.max(initial=0) < MAXROUNDS
        parity = occ % 2
        paired = (2 * rnd + 1) < kcnt         # partner exists
        typ = (~paired).astype(np.int64)      # 0 = paired, 1 = unpaired
        # a row has exactly one unpaired edge per chunk (its last odd
        # occurrence), so all unpaired tokens of a chunk are distinct:
        # merge them into a single round-0 group -> fewer, fuller calls
        rnd = np.where(typ == 1, 0, rnd)
        # order: (chunk, round, type, parity, row) -> within a paired group the
        # parity-0 run and parity-1 run list the same rows in the same order
        o2 = np.lexsort((r, parity, typ, rnd, ch))
        r, co, v, ch, rnd, parity, typ = (
            a[o2] for a in (r, co, v, ch, rnd, parity, typ))
        for k in range(NCHUNK):
            for t in range(2):
                mk = (ch == k) & (typ == t)
                sizes_all[c, k, :, t] += np.bincount(
                    rnd[mk], minlength=MAXROUNDS)
        per_core.append((r, co % CHUNK, v, ch, rnd, parity, typ))

    caps = np.zeros((NCHUNK, MAXROUNDS, 2), np.int64)
    caps[:, :, 0] = ((sizes_all[:, :, :, 0].max(axis=0) + 255) // 256) * 256
    caps[:, :, 1] = ((sizes_all[:, :, :, 1].max(axis=0) + 127) // 128) * 128
    T = int(caps.sum())

    # static call list: (chunk, dram_offset, n_edges, type)
    calls = []
    off = 0
    for k in range(NCHUNK):
        for j in range(MAXROUNDS):
            for t in range(2):
                cap = int(caps[k, j, t])
                tile_sz = 2048 if t == 0 else TILE_E
                for t0 in range(0, cap, tile_sz):
                    calls.append((k, off + t0, min(tile_sz, cap - t0), t))
                off += cap

    gidx_w = np.zeros((NCORES, P, T // 16), np.int16)
    sp_w = np.zeros((NCORES, P, T // 2 // 16), np.int16)   # paired: idx per pair
    su_w = np.zeros((NCORES, P, T // 16), np.int16)        # unpaired idx
    sval_w = np.zeros((NCORES, P, T // 128), np.float32)
    offs = np.concatenate([[0], np.cumsum(caps.reshape(-1))]).astype(np.int64)

    for c in range(NCORES):
        r, coi, v, ch, rnd, parity, typ = per_core[c]
        gi = np.zeros(T, np.int16)
        sp = np.full(T // 2, 12600, np.int16)   # paired dump row (1KB units)
        su = np.full(T, 25200, np.int16)        # unpaired dump (512B units)
        sv = np.zeros(T, np.float32)
        s = 0
        gidx = 0
        for k in range(NCHUNK):
            for j in range(MAXROUNDS):
                for t in range(2):
                    d0 = int(offs[gidx]); gidx += 1
                    nn = int(sizes_all[c, k, j, t])
                    if t == 1:
                        gi[d0:d0 + nn] = coi[s:s + nn]
                        su[d0:d0 + nn] = r[s:s + nn] * 2   # parity always 0
                        sv[d0:d0 + nn] = v[s:s + nn]
                        s += nn
                        continue
                    # paired: nn edges = 2*npair; first npair are parity 0
                    npair = nn // 2
                    mseq = np.arange(npair)
                    pos0 = d0 + (mseq // 128) * 256 + (mseq % 128)
                    pos1 = pos0 + 128
                    gi[pos0] = coi[s:s + npair]
                    sv[pos0] = v[s:s + npair]
                    gi[pos1] = coi[s + npair:s + nn]
                    sv[pos1] = v[s + npair:s + nn]
                    sp[(d0 // 2) + mseq] = r[s:s + npair]
                    s += nn
        gidx_w[c] = np.tile(gi.reshape(-1, 16).T, (8, 1))
        sp_w[c] = np.tile(sp.reshape(-1, 16).T, (8, 1))
        su_w[c] = np.tile(su.reshape(-1, 16).T, (8, 1))
        sval_w[c] = sv.reshape(-1, 128).T

    return tuple(calls), gidx_w, sp_w, su_w, sval_w


def _build(calls):
    from concourse import bass, mybir, tile, bacc
    from concourse.masks import make_identity

    f32 = mybir.dt.float32
    i16 = mybir.dt.int16
    T = max(e0 + n for _, e0, n, _t in calls)

    nc = bacc.Bacc("TRN2", target_bir_lowering=False, debug=False,
                   num_swdge_queues=4)

    x_d = nc.dram_tensor("x", [N_NODES, D], f32, kind="ExternalInput")
    xT_d = nc.dram_tensor("xlocT", [D, RPC], f32, kind="ExternalInput")
    w_d = nc.dram_tensor("W", [2 * D, OUT], f32, kind="ExternalInput")
    gidx_d = nc.dram_tensor("gidx", [P, T // 16], i16, kind="ExternalInput")
    sp_d = nc.dram_tensor("spair", [P, T // 2 // 16], i16, kind="ExternalInput")
    su_d = nc.dram_tensor("sunp", [P, T // 16], i16, kind="ExternalInput")
    sval_d = nc.dram_tensor("svals", [P, T // 128], f32, kind="ExternalInput")
    out_d = nc.dram_tensor("out", [RPC, OUT], f32, kind="ExternalOutput")
    h_d = nc.dram_tensor("h_acc", [H_PAD, 2 * D], f32)   # 1KB rows, 2 stripes

    relu = mybir.ActivationFunctionType.Relu

    with tile.TileContext(nc) as tc:
        with tc.tile_pool(name="const", bufs=1) as constp, \
             tc.tile_pool(name="mess", bufs=6) as messp, \
             tc.tile_pool(name="meta", bufs=6) as metap, \
             tc.tile_pool(name="dense", bufs=4) as densep, \
             tc.tile_pool(name="psum", bufs=2, space="PSUM") as psump:

            ident = constp.tile([P, P], f32)
            make_identity(nc, ident[:])
            w1 = constp.tile([D, OUT], f32)
            nc.sync.dma_start(out=w1[:], in_=w_d[:D, :])
            w2 = constp.tile([D, OUT], f32)
            nc.sync.dma_start(out=w2[:], in_=w_d[D:, :])
            # preload the whole xlocT arena (tail's x-side inputs, h-independent)
            xta = constp.tile([P, RPC], f32)
            for c0 in range(0, RPC, 2500):
                nc.scalar.dma_start(out=xta[:, c0:c0 + 2500],
                                    in_=xT_d[:, c0:c0 + 2500])

            # zero the striped h accumulator (12800 rows x 1KB)
            zblk = constp.tile([P, 4, 2 * D], f32)
            nc.vector.memset(zblk[:], 0.0)
            for b in range(H_PAD // 512):
                dst = h_d[b * 512:(b + 1) * 512, :].rearrange(
                    "(a p) d -> p a d", p=P)
                nc.scalar.dma_start(out=dst, in_=zblk[:])

            # ---- SpMM phase: gather -> scale -> scatter-add ----
            h_unp = h_d[:].rearrange("r (s d) -> (r s) d", s=2)
            qrr = 0
            for k, e0, n, typ in calls:
                ns = n // 128
                x_chunk = x_d[k * CHUNK:(k + 1) * CHUNK, :]
                gi = metap.tile([P, 2048 // 16], i16, tag="gi")
                nc.sync.dma_start(
                    out=gi[:, :n // 16],
                    in_=gidx_d[:, e0 // 16:(e0 + n) // 16])
                mv = messp.tile([P, 2048 // 128, D], f32, tag="mess")
                for sub in range(0, n, 1024):
                    gn = min(1024, n - sub)
                    nc.gpsimd.dma_gather(
                        mv[:, sub // 128:(sub + gn) // 128, :], x_chunk,
                        gi[:, sub // 16:(sub + gn) // 16], gn, gn, D,
                        queue_num=1 + (qrr % 3))
                    qrr += 1
                sv = metap.tile([P, 2048 // 128], f32, tag="sv")
                nc.sync.dma_start(
                    out=sv[:, :ns],
                    in_=sval_d[:, e0 // 128:(e0 + n) // 128])
                nc.vector.tensor_tensor(
                    out=mv[:, :ns, :], in0=mv[:, :ns, :],
                    in1=sv[:, :ns, None].to_broadcast([P, ns, D]),
                    op=mybir.AluOpType.mult)
                if typ == 0:
                    npr = n // 2
                    si = metap.tile([P, 2048 // 2 // 16], i16, tag="sip")
                    nc.scalar.dma_start(
                        out=si[:, :npr // 16],
                        in_=sp_d[:, e0 // 2 // 16:(e0 // 2 + npr) // 16])
                    nc.gpsimd.dma_scatter_add(
                        h_d[:],
                        mv[:, :ns, :].rearrange("p a d -> p (a d)").rearrange(
                            "p (a d) -> p a d", d=2 * D),
                        si[:, :npr // 16], npr, npr, 2 * D)
                else:
                    si = metap.tile([P, TILE_E // 16], i16, tag="si")
                    nc.scalar.dma_start(
                        out=si[:, :n // 16],
                        in_=su_d[:, e0 // 16:(e0 + n) // 16])
                    nc.gpsimd.dma_scatter_add(
                        h_unp, mv[:, :ns, :], si[:, :n // 16], n, n, D)

            # ---- dense head: out = relu(h @ W1 + x @ W2) ----
            # batched: 4 row-groups (512 rows) per h-load / out-store DMA
            for b in range((RPC + 511) // 512):
                r0 = b * 512
                rb = min(512, RPC - r0)
                nsub = (rb + P - 1) // P
                hl4 = densep.tile([P, 4, 2 * D], f32, tag="hl")
                nc.sync.dma_start(
                    out=hl4[:, :nsub, :],
                    in_=h_d[r0:r0 + nsub * P, :].rearrange(
                        "(a p) d -> p a d", p=P))
                ob4 = densep.tile([P, 4, OUT], f32, tag="ob")
                for a in range(nsub):
                    g0 = r0 + a * P
                    rsz = min(P, RPC - g0)
                    hb = densep.tile([P, D], f32, tag="hb")
                    nc.vector.tensor_add(out=hb[:rsz, :],
                                         in0=hl4[:rsz, a, :D],
                                         in1=hl4[:rsz, a, D:])
                    pt = psump.tile([P, P], f32, tag="pt")
                    nc.tensor.transpose(pt[:, :rsz], hb[:rsz, :],
                                        ident[:rsz, :rsz])
                    hT = densep.tile([P, P], f32, tag="hT")
                    nc.vector.tensor_copy(hT[:, :rsz], pt[:, :rsz])
                    po = psump.tile([P, OUT], f32, tag="po")
                    nc.tensor.matmul(po[:rsz, :], hT[:, :rsz], w1[:],
                                     start=True, stop=False)
                    nc.tensor.matmul(po[:rsz, :], xta[:, g0:g0 + rsz], w2[:],
                                     start=False, stop=True)
                    nc.scalar.activation(ob4[:rsz, a, :], po[:rsz, :], relu)
                if rb == 512:
                    nc.scalar.dma_start(
                        out=out_d[r0:r0 + 512, :].rearrange(
                            "(a p) d -> p a d", p=P),
                        in_=ob4[:])
                else:
                    for a in range(nsub):
                        g0 = r0 + a * P
                        rsz = min(P, RPC - g0)
                        nc.scalar.dma_start(out=out_d[g0:g0 + rsz, :],
                                            in_=ob4[:rsz, a, :])

    nc.compile()
    return nc


def _get_nc(calls):
    nc = _compiled.get(calls)
    if nc is None:
        nc = _build(calls)
        _compiled[calls] = nc
    return nc


def _make_in_maps(x, W, calls, gidx_w, sp_w, su_w, sval_w):
    x = np.ascontiguousarray(np.asarray(x, np.float32))
    W = np.ascontiguousarray(np.asarray(W, np.float32))
    in_maps = []
    for c in range(NCORES):
        xloc = x[c * RPC:(c + 1) * RPC]
        in_maps.append({
            "x": x,
            "xlocT": np.ascontiguousarray(xloc.T),
            "W": W,
            "gidx": gidx_w[c],
            "spair": sp_w[c],
            "sunp": su_w[c],
            "svals": sval_w[c],
        })
    return in_maps


def _install_trace_shims():
    """Make trace=True work in this container: provide antenv.axon_hooks
    (ctypes NTFF profiling via the axon PJRT .so) and stub the artifact
    upload (no bucket access here)."""
    import contextlib
    import ctypes
    import types

    try:
        import antenv.axon_hooks  # noqa: F401
        has_hooks = True
    except ImportError:
        has_hooks = False
    if not has_hooks:
        so_path = "/opt/axon/libaxon_pjrt.so"
        lib = ctypes.CDLL(so_path)
        if hasattr(lib, "axon_start_nrt_profile"):
            lib.axon_start_nrt_profile.argtypes = [
                ctypes.POINTER(ctypes.c_int64), ctypes.c_size_t]
            lib.axon_start_nrt_profile.restype = ctypes.c_int64
            lib.axon_stop_nrt_profile.argtypes = [ctypes.c_char_p]
            lib.axon_stop_nrt_profile.restype = ctypes.c_int64

            @contextlib.contextmanager
            def _hook(output_dir, device_ids):
                import jax
                jax.devices()
                if device_ids:
                    ids = (ctypes.c_int64 * len(device_ids))(*device_ids)
                    rc = lib.axon_start_nrt_profile(ids, len(device_ids))
                else:
                    rc = lib.axon_start_nrt_profile(None, 0)
                if rc != 0:
                    raise RuntimeError(f"axon_start_nrt_profile rc={rc}")
                try:
                    yield
                finally:
                    n = lib.axon_stop_nrt_profile(str(output_dir).encode())
                    if n <= 0:
                        print(f"ntff profile: rc={n} (no files?) at {output_dir}")

            mod = types.ModuleType("antenv.axon_hooks")
            mod.get_axon_ntff_profile_hook = lambda: _hook
            mod.set_axon_ntff_profile_hook = lambda h: None
            sys.modules["antenv.axon_hooks"] = mod

    import concourse.bass_utils as bu
    bu.upload_artifacts = lambda tmpdir: f"local:{tmpdir}"


def _run(x, adj_rows, adj_cols, adj_vals, W, trace=False):
    from concourse.bass_utils import run_bass_kernel_spmd
    if trace:
        try:
            _install_trace_shims()
        except Exception as e:  # tracing is best-effort
            print("trace shim install failed:", e)
    calls, gidx_w, sp_w, su_w, sval_w = _prep(adj_rows, adj_cols, adj_vals)
    nc = _get_nc(calls)
    in_maps = _make_in_maps(x, W, calls, gidx_w, sp_w, su_w, sval_w)
    res = run_bass_kernel_spmd(nc, in_maps, list(range(NCORES)), trace=trace)
    out = np.concatenate([res.results[c]["out"] for c in range(NCORES)], axis=0)
    return out, res


def kernel(x, adj_rows, adj_cols, adj_vals, W):
    out, _ = _run(x, adj_rows, adj_cols, adj_vals, W, trace=False)
    return out
